# revision 1
# baseline (speedup 1.0000x reference)
"""Trainium2 Bass kernel for nn_ContrastiveLossV2 (8-core SPMD).

Reference computation:
    z = l2norm(concat([emb_i, emb_j]))          # [8192, 128]
    sim = z @ z.T                               # [8192, 8192]
    loss = mean((sim - class_pairs)**2)

Expanded square (no sim materialization):
    sum((sim-cp)^2) = sum(sim^2) - 2*sum(sim*cp) + sum(cp^2)
  * sum(sim^2)  = ||Z^T Z||_F^2  (per-core Gram over local rows, summed on
                  host in f64)
  * sum(sim*cp) = sum_{d,g} V[d,g] * zT[d,g],  V = Z_loc^T @ CP_loc
  * sum(cp^2)   = square+accumulate passes split across Act/Pool/DVE

Key layout decisions (vs the f32-streaming baseline):
  * class_pairs is cast f32 -> float8_e3m4 on the host during the sharding
    pass. The device math was already bf16 (all matmuls bf16/f8 with f32
    PSUM), so this only adds the fp8 quantization error (~1e-4 relative on
    the loss) while cutting the dominant HBM stream from 32MB to 8MB/core.
  * cp arrives pre-tiled in DMA-contiguous images: 9 column groups
    (7x1024 + 2x512 cols), each one [128, 8*W] tile = one HWDGE DMA with
    8KB/partition descriptors, all SBUF-resident so the stream never
    stalls on buffer recycling. Loads run on the SP hardware DGE queue,
    so the gpsimd engine spends nothing on descriptor generation and is
    free to run a share of the cp^2 squares.
  * Per-core row blocks are ROTATED into block 0..7 of the staged emb
    (and cp columns rotated to match), so one SPMD program serves all
    cores with no separate local-emb staging.
  * The staged emb is d-major ([128 rows, 128 d, n blocks]) in 4
    independent pieces: the first piece is exactly the local blocks so
    V matmuls can start ~4us in, and d-major layout makes the normalize
    multiply eligible for the DVE 2x packed mode (broadcast stride-0 axis
    is not innermost).
"""

import numpy as np

import concourse.bacc as bacc
import concourse.mybir as mybir
import concourse.tile as tile
from concourse.bass_utils import run_bass_kernel_spmd
from concourse.tile_rust import add_dep_helper

f32 = mybir.dt.float32
bf16 = mybir.dt.bfloat16
f8 = mybir.dt.float8e3
AF = mybir.ActivationFunctionType
OP = mybir.AluOpType

N_CORES = 8
N, D = 4096, 128
TWO_N = 2 * N                      # 8192
R_LOC = TWO_N // N_CORES           # 1024 rows per core
M_BLK = R_LOC // 128               # 8 local 128-row blocks
NB = TWO_N // 128                  # 64 total row blocks
NCH = 512                          # matmul free-dim chunk (one PSUM bank)
GROUP_W = [1024] * 7 + [512, 512]  # cp column-group widths (sum = 8192)
N_GRP = len(GROUP_W)
PIECES = [8, 18, 19, 19]           # normalize pipeline piece sizes (blocks)
EPS = 1e-12

# cp^2 square split per group: fraction of each group's 8*W columns that
# run on the scalar (Act) engine and the gpsimd (Pool) engine; the
# remainder goes to the vector (DVE) engine. The gpsimd engine has no
# fused square+accumulate instruction, so its share runs as a multiply
# followed by a full XYZWC reduce to one scalar (rate ~3.4ns/el vs
# 0.87 Act / 1.2 DVE) -- it still wins because it is otherwise idle.
# Tuned against TimelineSim.
ACT_FRAC = 0.70
POOL_FRAC = 0.20

# accumulator column layout: act cols | dve cols | pool scalar cols
# (valid on partition 0 only) | dot partials
A_COL, D_COL, P_COL, X_COL = 0, N_GRP, 2 * N_GRP, 3 * N_GRP
ACC_W = 3 * N_GRP + N_GRP

_cached = {}


def _offs(widths):
    offs, o = [], 0
    for w in widths:
        offs.append(o)
        o += w
    return offs


GRP_OFF = _offs(GROUP_W)
PIECE_OFF = _offs(PIECES)


def _build_module():
    nc = bacc.Bacc("TRN2", target_bir_lowering=False, debug=False,
                   num_devices=N_CORES)

    # staged inputs (host pre-arranged, see kernel()):
    #  embsP: [128, 128, nb] bf16, d-major: [row-in-block, d, block];
    #         global row-block of staged block k is (8c+k)%64
    #  cpst:  [128, 65536] f8; per partition, concatenation over groups of
    #         the [8, W] row-block x column slab for that partition
    embs_t = [nc.dram_tensor(f"embs{i}", [128, D, nb], bf16,
                             kind="ExternalInput")
              for i, nb in enumerate(PIECES)]
    cpst = nc.dram_tensor("cpst", [128, M_BLK * TWO_N], f8,
                          kind="ExternalInput")
    ident = nc.dram_tensor("ident", [128, 128], bf16, kind="ExternalInput")
    out_g = nc.dram_tensor("out_g", [128, 128], f32, kind="ExternalOutput")
    out_acc = nc.dram_tensor("out_acc", [128, ACC_W], f32,
                             kind="ExternalOutput")

    with tile.TileContext(nc) as tc:
        with (
            tc.tile_pool(name="const", bufs=1) as const_pool,
            tc.tile_pool(name="persist", bufs=1) as persist,
            tc.tile_pool(name="sq", bufs=2) as sq_pool,
            tc.tile_pool(name="norm", bufs=12) as norm_pool,
            tc.tile_pool(name="cpt", bufs=9) as cp_pool,
            tc.tile_pool(name="tmp", bufs=4) as tmp_pool,
            tc.tile_pool(name="sqa", bufs=2) as sqa_pool,
            tc.tile_pool(name="sqp", bufs=2) as sqp_pool,
            tc.tile_pool(name="sqd", bufs=2) as sqd_pool,
            tc.tile_pool(name="psv", bufs=2, space="PSUM") as psv_pool,
            tc.tile_pool(name="pst", bufs=2, space="PSUM") as pst_pool,
            tc.tile_pool(name="psg", bufs=1, space="PSUM") as psg_pool,
        ):
            stag = [persist.tile([128, D, nb], bf16, name=f"stag{i}")
                    for i, nb in enumerate(PIECES)]
            z = [persist.tile([128, D, nb], bf16, name=f"z{i}")
                 for i, nb in enumerate(PIECES)]
            zT = persist.tile([128, TWO_N], bf16)       # z transposed
            acc = persist.tile([128, ACC_W], f32)

            ident_sb = const_pool.tile([128, 128], bf16)

            # ---- input DMAs, all on the SP hardware DGE queue, in a
            # strict issue order: local piece, ident, cp group 0, the
            # remaining pieces, cp groups 1-8.
            dmas = []
            dmas.append(nc.sync.dma_start(out=stag[0][:], in_=embs_t[0][:]))
            dmas.append(nc.sync.dma_start(out=ident_sb[:], in_=ident[:]))

            cpts = []
            for g, gw in enumerate(GROUP_W):
                cpt = cp_pool.tile([128, M_BLK * gw], f8, tag="cpt",
                                   name="cpt",
                                   padded_shape=[128, M_BLK * GROUP_W[0]])
                off = M_BLK * GRP_OFF[g]
                dmas.append(nc.sync.dma_start(
                    out=cpt[:], in_=cpst[:, off:off + M_BLK * gw]))
                cpts.append(cpt)
                if g == 0:
                    for q in range(1, len(PIECES)):
                        dmas.append(nc.sync.dma_start(
                            out=stag[q][:], in_=embs_t[q][:]))
            for a, b in zip(dmas, dmas[1:]):
                add_dep_helper(b.ins, a.ins, False, "input DMA queue order")


            def do_norm(q):
                nb = PIECES[q]
                sqt = sq_pool.tile([128, D, nb], bf16, tag="sqt", name="sqt",
                                   padded_shape=[128, D, max(PIECES)])
                nc.vector.tensor_tensor(sqt[:], stag[q][:], stag[q][:],
                                        op=OP.mult)
                nsq = norm_pool.tile([128, nb], bf16, tag="nsq", name="nsq")
                with nc.allow_low_precision(
                        reason="norm^2 in bf16: 0.4% rel err on the row "
                               "norm is ~1e-4 on the loss, gate is 2e-2"):
                    nc.vector.tensor_reduce(
                        nsq[:], sqt[:].rearrange("q d n -> q n d"),
                        axis=mybir.AxisListType.X, op=OP.add)
                nrm = norm_pool.tile([128, nb], f32, tag="nrm", name="nrm")
                nc.scalar.activation(nrm[:], nsq[:], AF.Sqrt)
                nc.vector.tensor_scalar_max(nrm[:], nrm[:], EPS)
                w = norm_pool.tile([128, nb], bf16, tag="w", name="w")
                with nc.allow_low_precision(
                        reason="1/norm in bf16, same error budget as above"):
                    nc.vector.reciprocal(w[:], nrm[:])
                w_b = w[:].rearrange("q n -> q () n").broadcast_to(
                    [128, D, nb])
                nc.vector.tensor_tensor(z[q][:], stag[q][:], w_b, op=OP.mult)

            def do_transposes(q):
                nb = PIECES[q]
                for j0 in range(0, nb, 8):
                    jn = min(8, nb - j0)
                    ps8 = pst_pool.tile([128, 8, 128], bf16, tag="ps8",
                                        name="ps8")
                    for dlt in range(jn):
                        nc.tensor.transpose(ps8[:, dlt, :],
                                            z[q][:, :, j0 + dlt],
                                            ident_sb[:])
                    c0 = (PIECE_OFF[q] + j0) * 128
                    nc.vector.tensor_copy(
                        zT[:, c0:c0 + jn * 128]
                        .rearrange("q (n p) -> q n p", n=jn),
                        ps8[:, :jn, :])

            def do_gram():
                g_ps = psg_pool.tile([128, 128], f32)
                for m in range(M_BLK):
                    nc.tensor.matmul(g_ps[:], lhsT=z[0][:, :, m],
                                     rhs=z[0][:, :, m],
                                     start=(m == 0), stop=(m == M_BLK - 1))
                g_sb = tmp_pool.tile([128, 128], f32, tag="gsb", name="gsb")
                nc.scalar.copy(g_sb[:], g_ps[:])
                nc.sync.dma_start(out=out_g[:], in_=g_sb[:])

            def do_group(g):
                gw = GROUP_W[g]
                cpt = cpts[g]
                cpv = cpt[:].rearrange("q (m w) -> q m w", m=M_BLK)
                col0 = GRP_OFF[g]
                n_k = gw // NCH
                # V matmuls: all chunks of the group into one (multi-bank)
                # PSUM tile, then a single fused dot against zT
                ps = psv_pool.tile([128, n_k, NCH], f32, tag="psv",
                                   name="psv", padded_shape=[128, 2, NCH])
                for k in range(n_k):
                    for m in range(M_BLK):
                        nc.tensor.matmul(
                            ps[:, k, :], lhsT=z[0][:, :, m],
                            rhs=cpv[:, m, k * NCH:(k + 1) * NCH],
                            start=(m == 0), stop=(m == M_BLK - 1))
                xj = tmp_pool.tile([128, gw], bf16, tag="xj", name="xj",
                                   padded_shape=[128, GROUP_W[0]])
                nc.vector.scalar_tensor_tensor(
                    out=xj[:], in0=ps[:].rearrange("q k c -> q (k c)"),
                    scalar=1.0, in1=zT[:, col0:col0 + gw],
                    op0=OP.mult, op1=OP.mult,
                    accum_out=acc[:, X_COL + g:X_COL + g + 1])
                # squares, split across Act / Pool / DVE by column range
                tot = M_BLK * gw
                a_end = (int(tot * ACT_FRAC) // 16) * 16
                p_end = a_end + (int(tot * POOL_FRAC) // 16) * 16
                max_tot = M_BLK * GROUP_W[0]
                pads = [(int(max_tot * ACT_FRAC) // 16) * 16,
                        (int(max_tot * POOL_FRAC) // 16) * 16]
                pads.append(max_tot - pads[0] - pads[1])
                sja = sqa_pool.tile([128, a_end], f8, tag="sj", name="sja",
                                    padded_shape=[128, pads[0]])
                nc.scalar.activation(sja[:], cpt[:, 0:a_end], AF.Square,
                                     accum_out=acc[:, A_COL + g:A_COL + g + 1])
                sjp = sqp_pool.tile([128, p_end - a_end], bf16, tag="sj",
                                    name="sjp", padded_shape=[128, pads[1]])
                nc.gpsimd.tensor_tensor(sjp[:], cpt[:, a_end:p_end],
                                        cpt[:, a_end:p_end], op=OP.mult)
                nc.gpsimd.tensor_reduce(acc[0:1, P_COL + g:P_COL + g + 1],
                                        sjp[:], axis=mybir.AxisListType.XYZWC,
                                        op=OP.add)
                sjd = sqd_pool.tile([128, tot - p_end], f8, tag="sj",
                                    name="sjd", padded_shape=[128, pads[2]])
                nc.vector.scalar_tensor_tensor(
                    out=sjd[:], in0=cpt[:, p_end:tot], scalar=1.0,
                    in1=cpt[:, p_end:tot], op0=OP.mult, op1=OP.mult,
                    accum_out=acc[:, D_COL + g:D_COL + g + 1])

            # interleaved issue: each norm piece, then its transposes, then
            # the cp groups whose zT columns are fully transposed
            do_norm(0)
            do_transposes(0)
            do_gram()
            do_group(0)
            grp_schedule = {1: [1, 2], 2: [3, 4], 3: [5, 6, 7, 8]}
            for q in (1, 2, 3):
                do_norm(q)
                do_transposes(q)
                for g in grp_schedule[q]:
                    do_group(g)

            nc.sync.dma_start(out=out_acc[:], in_=acc[:])

    nc.compile()
    return nc


def _get_module():
    if "nc" not in _cached:
        _cached["nc"] = _build_module()
    return _cached["nc"]


def kernel(emb_i, emb_j, class_pairs, _return_raw=False, _trace=False):
    import ml_dtypes

    emb_i = np.ascontiguousarray(emb_i, dtype=np.float32)
    emb_j = np.ascontiguousarray(emb_j, dtype=np.float32)
    class_pairs = np.ascontiguousarray(class_pairs, dtype=np.float32)
    ident = np.eye(128, dtype=ml_dtypes.bfloat16)

    emb = np.concatenate([emb_i, emb_j], axis=0)          # [8192, 128]
    emb_blocks = emb.reshape(NB, 128, D).astype(ml_dtypes.bfloat16)
    cp_f8 = class_pairs.astype(ml_dtypes.float8_e3m4)     # host-side cast

    nc = _get_module()
    in_maps = []
    for c in range(N_CORES):
        order = [(M_BLK * c + k) % NB for k in range(NB)]
        rot_blocks = emb_blocks[order]                    # [64, 128, 128]

        r0 = c * R_LOC
        rows = cp_f8[r0:r0 + R_LOC]                       # [1024, 8192]
        s = r0 % TWO_N
        rot = np.concatenate([rows[:, s:], rows[:, :s]], axis=1)
        rb = rot.reshape(M_BLK, 128, TWO_N)               # [8, 128, 8192]
        parts = []
        for gw, off in zip(GROUP_W, GRP_OFF):
            parts.append(rb[:, :, off:off + gw].transpose(1, 0, 2)
                         .reshape(128, M_BLK * gw))
        cp_st = np.ascontiguousarray(np.concatenate(parts, axis=1))

        in_map = {"cpst": cp_st, "ident": ident}
        for i, (nb, po) in enumerate(zip(PIECES, PIECE_OFF)):
            in_map[f"embs{i}"] = np.ascontiguousarray(
                rot_blocks[po:po + nb].transpose(1, 2, 0))  # [128, 128, nb]
        in_maps.append(in_map)

    res = run_bass_kernel_spmd(nc, in_maps, list(range(N_CORES)),
                               trace=_trace)

    G = np.zeros((128, 128), dtype=np.float64)
    sum_cp2 = 0.0
    cross = 0.0
    for c in range(N_CORES):
        G += res.results[c]["out_g"].astype(np.float64)
        out = res.results[c]["out_acc"].astype(np.float64)
        sum_cp2 += out[:, A_COL:P_COL].sum()        # act + dve partials
        sum_cp2 += out[0, P_COL:X_COL].sum()        # pool scalars (row 0)
        cross += out[:, X_COL:].sum()
    sum_sim2 = float((G * G).sum())
    loss = (sum_sim2 - 2.0 * cross + sum_cp2) / float(TWO_N * TWO_N)
    out = np.asarray(loss, dtype=np.float32)
    if _return_raw:
        return out, res
    return out

